# revision 5
# baseline (speedup 1.0000x reference)
"""Trainium2 Bass kernel for 16-head MHA (B=4, L=2048, D=1024).

Sharding: 8 cores = 4 batches x 2 head-groups (8 heads each).
Per core (batch b, head-group g):
  phase A: QKV projection.  qT/kT produced in [feature, L] layout
           (W1 tile stationary, xT moving); v produced in natural
           [kpos, feature] layout (xT stationary, W1v moving) with a
           ones-column appended per head (for the softmax denominator).
           q weights/bias pre-scaled by 1/sqrt(dk)=0.125 on host (exact).
  phase B: per head pair: scores sT = kT.T @ qT  (K=64 row-packed pairs),
           e = exp(sT) (no max subtraction needed: |scores| <~ 7),
           e *= maskT,  ctxT_aug = v_aug.T @ e accumulated over kpos.
           Row 64 of ctxT_aug is sum(e) -> denominator (+ host-side
           zero-count, matching torch's float_info.min==0.0-in-fp32 fill
           semantics), reciprocal, broadcast, normalize.
  phase C: y_partial = ctx @ W2 (row-slice of W2); host sums the two
           head-group partials + b2.
"""

import sys

if "/opt/trn_rl_repo" not in sys.path:
    sys.path.insert(0, "/opt/trn_rl_repo")

from contextlib import ExitStack

import numpy as np

import concourse.bass as bass  # noqa: F401  (AP types used implicitly)
import concourse.tile as tile
from concourse import bacc, bass_utils, mybir

f32 = mybir.dt.float32
AF = mybir.ActivationFunctionType

B, L, D, H = 4, 2048, 1024, 16
NH = 8           # heads per core
DK = 64
F = NH * DK      # 512 q/k/v features per core
NKD = D // 128   # 8 contraction tiles over D
NLB = L // 512   # 4 L blocks (phase A)
NQB = L // 512   # 4 q blocks (phase B)
NKB = L // 128   # 16 kpos blocks

LAST_EXEC_NS = None
LAST_RESULTS = None


def _body(ctx, tc, nc, t):
    small = ctx.enter_context(tc.tile_pool(name="small", bufs=1))
    kvpool = ctx.enter_context(tc.tile_pool(name="kv", bufs=1))
    dpool = ctx.enter_context(tc.tile_pool(name="dram", bufs=1, space="DRAM"))
    psA = ctx.enter_context(tc.tile_pool(name="psA", bufs=2, space="PSUM"))
    psS = ctx.enter_context(tc.tile_pool(name="psS", bufs=4, space="PSUM"))
    psC = ctx.enter_context(tc.tile_pool(name="psC", bufs=1, space="PSUM"))

    b1qk_sb = small.tile([128, 8], f32)
    cnt0_sb = small.tile([1, L], f32)
    nc.sync.dma_start(b1qk_sb[:], t.b1qk[:])
    nc.sync.dma_start(cnt0_sb[:], t.cnt0[:])

    kT = [kvpool.tile([128, L], f32, tag=f"kT{i}", name=f"kT{i}") for i in range(4)]
    vsb = [kvpool.tile([128, NH * 65], f32, tag=f"v{i}", name=f"v{i}") for i in range(NKB)]
    qTd = dpool.tile([F, L], f32)

    # ---------------- phase A: QKV projection ----------------
    with tc.tile_pool(name="phA", bufs=2) as pA, tc.tile_pool(name="phA1", bufs=1) as pA1:
        w1v_sb = [pA1.tile([128, F], f32, tag=f"w1v{kd}", name=f"w1v{kd}") for kd in range(NKD)]
        for kd in range(NKD):
            nc.sync.dma_start(w1v_sb[kd][:], t.w1v[kd * 128:(kd + 1) * 128, :])
        b1v_sb = pA1.tile([128, F], f32)
        nc.sync.dma_start(b1v_sb[:], t.b1v[:])
        b1vv = b1v_sb[:].rearrange("p (h c) -> p h c", c=DK)

        for lb in range(NLB):
            ls = slice(lb * 512, (lb + 1) * 512)
            xt = [pA.tile([128, 512], f32, tag=f"xt{kd}", name=f"xt{kd}") for kd in range(NKD)]
            w1qk_t = [pA1.tile([128, 2 * F], f32, tag=f"w1qk{kd}", name=f"w1qk{kd}") for kd in range(NKD)]
            for kd in range(NKD):
                nc.sync.dma_start(xt[kd][:], t.xT[kd * 128:(kd + 1) * 128, ls])
                nc.sync.dma_start(w1qk_t[kd][:], t.w1qk[kd * 128:(kd + 1) * 128, :])
            # q/k: out = W1qk_tile.T @ xT -> [features, L-block]
            for fb in range(8):
                ps = psA.tile([128, 512], f32)
                for kd in range(NKD):
                    nc.tensor.matmul(
                        ps[:], w1qk_t[kd][:, fb * 128:(fb + 1) * 128], xt[kd][:],
                        start=(kd == 0), stop=(kd == NKD - 1))
                if fb < 4:  # q features -> spill to DRAM (reloaded per q-block)
                    qt = pA.tile([128, 512], f32, tag="qspill")
                    nc.vector.tensor_scalar_add(qt[:], ps[:], b1qk_sb[:, fb:fb + 1])
                    nc.sync.dma_start(qTd[fb * 128:(fb + 1) * 128, ls], qt[:])
                else:  # k features -> resident
                    nc.vector.tensor_scalar_add(kT[fb - 4][:, ls], ps[:], b1qk_sb[:, fb:fb + 1])
            # v: out = xT_tile.T @ W1v -> [kpos-block, v features], 65-strided + ones col
            for j in range(4):
                kb = lb * 4 + j
                ps = psA.tile([128, 512], f32)
                for kd in range(NKD):
                    nc.tensor.matmul(
                        ps[:], xt[kd][:, j * 128:(j + 1) * 128], w1v_sb[kd][:],
                        start=(kd == 0), stop=(kd == NKD - 1))
                vv = vsb[kb][:].rearrange("p (h c) -> p h c", h=NH)
                nc.vector.tensor_add(
                    vv[:, :, 0:DK], ps[:].rearrange("p (h c) -> p h c", c=DK), b1vv)
                nc.vector.memset(vv[:, :, DK:DK + 1], 1.0)

    # ---------------- phase B (attention) + phase C (out proj) ----------------
    with tc.tile_pool(name="phB", bufs=1) as pB, \
         tc.tile_pool(name="phB2", bufs=2) as pB2, \
         tc.tile_pool(name="phBe", bufs=6) as pBe, \
         tc.tile_pool(name="phBs", bufs=2) as pBs, \
         tc.tile_pool(name="phBy", bufs=4) as pBy:
        w2_sb = [pB.tile([128, D], f32, tag=f"w2{f}", name=f"w2{f}") for f in range(4)]
        for f in range(4):
            nc.sync.dma_start(w2_sb[f][:], t.w2[f * 128:(f + 1) * 128, :])

        for qb in range(NQB):
            qs = slice(qb * 512, (qb + 1) * 512)
            qTq = [pB2.tile([128, 512], f32, tag=f"q{f}", name=f"q{f}") for f in range(4)]
            for f in range(4):
                nc.sync.dma_start(qTq[f][:], qTd[f * 128:(f + 1) * 128, qs])
            mT = [pB.tile([128, 512], f32, tag=f"m{kb}", name=f"m{kb}") for kb in range(NKB)]
            for kb in range(NKB):
                nc.sync.dma_start(mT[kb][:], t.maskT[kb * 128:(kb + 1) * 128, qs])
            ctxq = [pB2.tile([128, 512], f32, tag=f"ctx{f}", name=f"ctx{f}") for f in range(4)]

            for hp in range(4):
                cps = [psC.tile([65, 512], f32, name=f"cps{_i}") for _i in range(2)]
                for kb in range(NKB):
                    es = []
                    for i in range(2):  # row-packed head pair (K=64 at partitions 0/64)
                        pr = slice(i * 64, (i + 1) * 64)
                        sp = psS.tile([128, 512], f32)
                        nc.tensor.matmul(
                            sp[:], kT[hp][pr, kb * 128:(kb + 1) * 128], qTq[hp][pr, :])
                        e = pBe.tile([128, 512], f32, tag="e")
                        nc.scalar.activation(e[:], sp[:], AF.Exp)
                        nc.vector.tensor_mul(e[:], e[:], mT[kb][:])
                        es.append(e)
                    for i in range(2):
                        h = 2 * hp + i
                        nc.tensor.matmul(
                            cps[i][:], vsb[kb][:, h * 65:h * 65 + 65], es[i][:],
                            start=(kb == 0), stop=(kb == NKB - 1))
                for i in range(2):
                    # denominator: ctx row 64 (=sum e) + zero-count, then 1/x
                    den = pBs.tile([1, 512], f32, tag="den")
                    nc.vector.tensor_add(den[:], cps[i][64:65, :], cnt0_sb[0:1, qs])
                    rec = pBs.tile([1, 512], f32, tag="rec")
                    nc.vector.reciprocal(rec[:], den[:])
                    rb = pBs.tile([64, 512], f32, tag="rb")
                    nc.gpsimd.partition_broadcast(rb[:], rec[:])
                    nc.vector.tensor_mul(
                        ctxq[hp][i * 64:(i + 1) * 64, :], cps[i][0:64, :], rb[:])

            # phase C for this q-block: y = ctxT.T @ W2
            for lb2 in range(4):
                row0 = qb * 512 + lb2 * 128
                for nb in range(2):
                    ps = psA.tile([128, 512], f32)
                    for f in range(4):
                        nc.tensor.matmul(
                            ps[:], ctxq[f][:, lb2 * 128:(lb2 + 1) * 128],
                            w2_sb[f][:, nb * 512:(nb + 1) * 512],
                            start=(f == 0), stop=(f == 3))
                    yt = pBy.tile([128, 512], f32, tag="y")
                    nc.vector.tensor_copy(yt[:], ps[:])
                    nc.sync.dma_start(t.y[row0:row0 + 128, nb * 512:(nb + 1) * 512], yt[:])


def build_nc():
    nc = bacc.Bacc(None, target_bir_lowering=False)

    class T:
        pass

    t = T()
    t.xT = nc.dram_tensor("xT", [D, L], f32, kind="ExternalInput")
    t.maskT = nc.dram_tensor("maskT", [L, L], f32, kind="ExternalInput")
    t.w1qk = nc.dram_tensor("w1qk", [D, 2 * F], f32, kind="ExternalInput")
    t.w1v = nc.dram_tensor("w1v", [D, F], f32, kind="ExternalInput")
    t.b1qk = nc.dram_tensor("b1qk", [128, 8], f32, kind="ExternalInput")
    t.b1v = nc.dram_tensor("b1v", [128, F], f32, kind="ExternalInput")
    t.w2 = nc.dram_tensor("w2", [F, D], f32, kind="ExternalInput")
    t.cnt0 = nc.dram_tensor("cnt0", [1, L], f32, kind="ExternalInput")
    t.y = nc.dram_tensor("y", [L, D], f32, kind="ExternalOutput")
    with tile.TileContext(nc) as tc:
        with ExitStack() as ctx:
            _body(ctx, tc, nc, t)
    nc.compile()
    return nc


def prep_in_maps(inputs, mask, W1, b1, W2):
    inputs = np.asarray(inputs, np.float32)
    mask = np.asarray(mask)
    W1 = np.asarray(W1, np.float32)
    b1 = np.asarray(b1, np.float32)
    W2 = np.asarray(W2, np.float32)
    scale = np.float32(0.125)  # 1/sqrt(DK), exact power of two

    per_batch = []
    for b in range(B):
        xT = np.ascontiguousarray(inputs[b].T)                       # [D, L]
        maskT = np.ascontiguousarray(mask[b].T.astype(np.float32))   # [kpos, q]
        cnt0 = (L - mask[b].sum(axis=1)).astype(np.float32).reshape(1, L)
        per_batch.append((xT, maskT, cnt0))

    per_group = []
    for g in range(2):
        w1q = W1[:, g * F:(g + 1) * F] * scale
        w1k = W1[:, D + g * F:D + (g + 1) * F]
        w1v = np.ascontiguousarray(W1[:, 2 * D + g * F:2 * D + (g + 1) * F])
        w1qk = np.ascontiguousarray(np.concatenate([w1q, w1k], axis=1))
        b1q = b1[g * F:(g + 1) * F] * scale
        b1k = b1[D + g * F:D + (g + 1) * F]
        b1qk = np.ascontiguousarray(np.concatenate([b1q, b1k]).reshape(8, 128).T)
        b1v = np.ascontiguousarray(
            np.broadcast_to(b1[2 * D + g * F:2 * D + (g + 1) * F], (128, F)))
        w2 = np.ascontiguousarray(W2[g * F:(g + 1) * F, :])
        per_group.append((w1qk, w1v, b1qk, b1v, w2))

    in_maps = []
    for c in range(8):
        b, g = c // 2, c % 2
        xT, maskT, cnt0 = per_batch[b]
        w1qk, w1v, b1qk, b1v, w2 = per_group[g]
        in_maps.append(dict(xT=xT, maskT=maskT, cnt0=cnt0, w1qk=w1qk,
                            w1v=w1v, b1qk=b1qk, b1v=b1v, w2=w2))
    return in_maps


_NC = None


def _get_nc():
    global _NC
    if _NC is None:
        _NC = build_nc()
    return _NC


def _ensure_ntff_shim():
    """Provide antenv.axon_hooks (absent in this image) so trace=True works.

    Replicates trn_boot's ctypes hook into libaxon_pjrt.so.
    """
    try:
        from antenv.axon_hooks import get_axon_ntff_profile_hook  # noqa: F401
        return
    except ImportError:
        pass
    import contextlib
    import ctypes
    import types

    import antenv

    so_path = "/opt/axon/libaxon_pjrt.so"
    lib = ctypes.CDLL(so_path)
    if not hasattr(lib, "axon_start_nrt_profile"):
        hook = None
    else:
        lib.axon_start_nrt_profile.argtypes = [
            ctypes.POINTER(ctypes.c_int64), ctypes.c_size_t]
        lib.axon_start_nrt_profile.restype = ctypes.c_int64
        lib.axon_stop_nrt_profile.argtypes = [ctypes.c_char_p]
        lib.axon_stop_nrt_profile.restype = ctypes.c_int64

        @contextlib.contextmanager
        def hook(output_dir, device_ids):
            import jax
            jax.devices()
            if device_ids:
                ids = (ctypes.c_int64 * len(device_ids))(*device_ids)
                rc = lib.axon_start_nrt_profile(ids, len(device_ids))
            else:
                rc = lib.axon_start_nrt_profile(None, 0)
            if rc != 0:
                raise RuntimeError(f"axon_start_nrt_profile rc={rc}")
            try:
                yield
            finally:
                n = lib.axon_stop_nrt_profile(str(output_dir).encode())
                print(f"profile: {n} file(s) written to {output_dir}",
                      file=sys.stderr)

    mod = types.ModuleType("antenv.axon_hooks")
    mod._hook = hook
    mod.get_axon_ntff_profile_hook = lambda: mod._hook
    mod.set_axon_ntff_profile_hook = lambda h: setattr(mod, "_hook", h)
    sys.modules["antenv.axon_hooks"] = mod
    antenv.axon_hooks = mod


def kernel(inputs, mask, W1, b1, W2, b2, trace=False):
    global LAST_EXEC_NS, LAST_RESULTS
    b2 = np.asarray(b2, np.float32)
    if trace:
        _ensure_ntff_shim()
    nc = _get_nc()
    in_maps = prep_in_maps(inputs, mask, W1, b1, W2)
    res = bass_utils.run_bass_kernel_spmd(
        nc, in_maps, core_ids=list(range(8)), trace=trace)
    LAST_RESULTS = res
    LAST_EXEC_NS = res.exec_time_ns
    out = np.empty((B, L, D), np.float32)
    for b in range(B):
        out[b] = res.results[2 * b]["y"] + res.results[2 * b + 1]["y"] + b2[None, :]
    return out
